# revision 1
# baseline (speedup 1.0000x reference)
"""Distributed Trainium2 Bass kernel for a causal single-head attention layer.

Problem shapes (hardcoded): N=4, S=T=2048, D=1024, f32 I/O.
  q = query @ Wq.T ; k = key @ Wk.T ; v = value @ Wv.T
  y = softmax(mask(q k^T / sqrt(D))) v

Sharding over 8 NeuronCores: core c -> (batch n = c//2, parity h = c%2).
Each core owns 8 interleaved 128-row query blocks (global block G = 2j+h,
j=0..7), which balances the causal (triangular) score workload between the
two cores of a batch. K/V projections are duplicated per batch pair --
cheaper than exchanging K/V through collectives on this chip.

Device compute is fp16 (TensorEngine runs 16-bit at 4x the fp32 rate) with
f32 PSUM accumulation; the host pre-transposes/casts inputs so no on-device
transposes are needed:
  qT[e,s] = (Wq/32)T-weights x qT-inputs, kT[e,t], v[t,e] projections
  ST[t,s] = kT.T @ qT per 128-wide t-tile g, only for g <= 2j+1 (causal skip)
  EST = exp(ST) * mask  (mask data taken from the real attn_mask input)
  y[s,e], sums[s] = EST.T @ [v | 1]  (ones-column gives softmax denominator)
  out = y * (1/sums)
"""

import numpy as np

from concourse import bass, mybir, tile, bacc
from concourse.bass_utils import run_bass_kernel_spmd

P = 128
N_BATCH = 4
S = 2048   # full query length
T = 2048   # key/value length
D = 1024   # model dim
SL = 1024  # per-core query rows
JB = SL // P   # 8 local s-blocks per core
GT = T // P    # 16 t-tiles
DO = D // P    # 8 outer tiles of the contraction dim
EO = D // P    # 8 outer tiles of the e dim
N_CORES = 8

_GRAPH_CACHE = {}


def _build_graph():
    if "nc" in _GRAPH_CACHE:
        return _GRAPH_CACHE["nc"]

    fp16 = mybir.dt.float16
    f32 = mybir.dt.float32

    nc = bacc.Bacc("TRN2", target_bir_lowering=False, debug=False,
                   num_devices=N_CORES)

    xqT_e = nc.dram_tensor("xqT", [D, SL], fp16, kind="ExternalInput")
    xkT_e = nc.dram_tensor("xkT", [D, T], fp16, kind="ExternalInput")
    xvT_e = nc.dram_tensor("xvT", [D, T], fp16, kind="ExternalInput")
    wqT_e = nc.dram_tensor("wqT", [D, D], fp16, kind="ExternalInput")
    wkT_e = nc.dram_tensor("wkT", [D, D], fp16, kind="ExternalInput")
    wvT_e = nc.dram_tensor("wvT", [D, D], fp16, kind="ExternalInput")
    mask_e = nc.dram_tensor("maskT", [GT, P, P], fp16, kind="ExternalInput")
    out_e = nc.dram_tensor("out", [SL, D], fp16, kind="ExternalOutput")

    xq_r = xqT_e.ap().rearrange("(o p) s -> p o s", p=P)
    xk_r = xkT_e.ap().rearrange("(o p) t -> p o t", p=P)
    xv_r = xvT_e.ap().rearrange("(o p) t -> p o t", p=P)
    wq_r = wqT_e.ap().rearrange("(o p) e -> p o e", p=P)
    wk_r = wkT_e.ap().rearrange("(o p) e -> p o e", p=P)
    wv_r = wvT_e.ap().rearrange("(o p) e -> p o e", p=P)

    with tile.TileContext(nc) as tc:
        with (
            tc.tile_pool(name="persist", bufs=1) as persist,
            tc.tile_pool(name="weights", bufs=2) as wpool,
            tc.tile_pool(name="xin", bufs=3) as xpool,
            tc.tile_pool(name="yout", bufs=2) as ypool,
        ):
            qT = persist.tile([P, EO, SL], fp16)       # [e-part, e-outer, s]
            kT = persist.tile([P, EO, T], fp16)        # [e-part, e-outer, t]
            vA = persist.tile([P, GT, D + 1], fp16)    # [t-part, t-outer, e+1]
            est = persist.tile([P, GT, SL], fp16)      # [t-part, t-outer, s]
            maskT = persist.tile([P, GT, P], fp16)     # [t-part, g, s-local]
            recip = persist.tile([P, JB], f32)

            # ---- Q projection: qT[e,s] = wqT.T @ xqT ----
            # First chunk in (o, m) order: accumulate all 8 e-tiles across 8
            # PSUM banks so the first matmul only needs the o=0 input slices.
            with tc.tile_pool(name="qpsum", bufs=8, space="PSUM") as qpsum:
                with nc.named_scope("projQ"):
                    wq = wpool.tile([P, DO, D], fp16, tag="w")
                    nc.scalar.dma_start(wq[:, 0, 0:P], wq_r[:, 0, 0:P])
                    nc.scalar.dma_start(wq[:, 0, P:D], wq_r[:, 0, P:D])
                    for o in range(1, DO):
                        nc.scalar.dma_start(wq[:, o, :], wq_r[:, o, :])
                    xqs = []
                    for sc in range(SL // 512):
                        xq = xpool.tile([P, DO, 512], fp16, tag="x", name=f"xq{sc}")
                        for o in range(DO):
                            if sc == 0 and o == 0:
                                nc.sync.dma_start(xq[:, 0, 0:256], xq_r[:, 0, 0:256])
                                nc.sync.dma_start(xq[:, 0, 256:512], xq_r[:, 0, 256:512])
                            else:
                                nc.sync.dma_start(
                                    xq[:, o, :], xq_r[:, o, 512 * sc:512 * (sc + 1)])
                        xqs.append(xq)
                    # chunk 0: (o, m) order. The o=0 pass only touches columns
                    # 0:256 (so the first matmul needs just the first 64KB DMA
                    # piece); exactly ONE start=True per PSUM bank clears it,
                    # and o=0's contribution to columns 256:512 is accumulated
                    # by a trailing matmul after the o=1..7 full-width passes.
                    pss = [qpsum.tile([P, 512], f32, tag="qp", name=f"qp0_{m2}")
                           for m2 in range(EO)]
                    for m in range(EO):
                        nc.tensor.matmul(
                            pss[m][:, 0:256], wq[:, 0, m * P:(m + 1) * P],
                            xqs[0][:, 0, 0:256],
                            start=True, stop=False, skip_group_check=True,
                        )
                    for o in range(1, DO):
                        for m in range(EO):
                            nc.tensor.matmul(
                                pss[m][:], wq[:, o, m * P:(m + 1) * P], xqs[0][:, o, :],
                                start=False, stop=False, skip_group_check=True,
                            )
                    for m in range(EO):
                        nc.tensor.matmul(
                            pss[m][:, 256:512], wq[:, 0, m * P:(m + 1) * P],
                            xqs[0][:, 0, 256:512],
                            start=False, stop=True, skip_group_check=True,
                        )
                        nc.any.tensor_copy(qT[:, m, 0:512], pss[m][:])
                    # chunk 1: (m, o) order -- psum slots recycle one at a time
                    for m in range(EO):
                        ps = qpsum.tile([P, 512], f32, tag="qp", name=f"qp1_{m}")
                        for o in range(DO):
                            nc.tensor.matmul(
                                ps[:], wq[:, o, m * P:(m + 1) * P], xqs[1][:, o, :],
                                start=(o == 0), stop=(o == DO - 1),
                            )
                        nc.any.tensor_copy(qT[:, m, 512:1024], ps[:])

            with (
                tc.tile_pool(name="ppsum", bufs=4, space="PSUM") as ppsum,
                tc.tile_pool(name="spsum", bufs=2, space="PSUM") as spsum,
            ):
                # ---- K projection: kT[e,t] = wkT.T @ xkT ----
                with nc.named_scope("projK"):
                    wk = wpool.tile([P, DO, D], fp16, tag="w")
                    for o in range(DO):
                        nc.scalar.dma_start(wk[:, o, :], wk_r[:, o, :])
                    for tck in range(T // 512):
                        xk = xpool.tile([P, DO, 512], fp16, tag="x")
                        for o in range(DO):
                            eng = nc.sync
                            eng.dma_start(
                                xk[:, o, :], xk_r[:, o, 512 * tck:512 * (tck + 1)])
                        for m in range(EO):
                            ps = ppsum.tile([P, 512], f32, tag="pp")
                            for o in range(DO):
                                nc.tensor.matmul(
                                    ps[:], wk[:, o, m * P:(m + 1) * P], xk[:, o, :],
                                    start=(o == 0), stop=(o == DO - 1),
                                )
                            nc.any.tensor_copy(kT[:, m, 512 * tck:512 * (tck + 1)], ps[:])

                # mask / ones-column setup (needed from phase A / B on)
                nc.scalar.dma_start(maskT[:], mask_e.ap().rearrange("g p s -> p g s"))
                nc.vector.memset(vA[:, :, D:D + 1], 1.0)

                # ---- scores + exp + mask, per t-tile g ----
                with nc.named_scope("scores"):
                    for g in range(GT):
                        j0 = g // 2
                        s0 = j0 * P
                        ncols = SL - s0
                        ps = spsum.tile([P, 1024], f32, tag="sp")
                        n_first = min(512, ncols)
                        for c in range(EO):
                            lhsT = kT[:, c, g * P:(g + 1) * P]
                            nc.tensor.matmul(
                                ps[:, 0:n_first], lhsT, qT[:, c, s0:s0 + n_first],
                                start=(c == 0), stop=(c == EO - 1),
                            )
                            if ncols > 512:
                                nc.tensor.matmul(
                                    ps[:, 512:ncols], lhsT, qT[:, c, s0 + 512:SL],
                                    start=(c == 0), stop=(c == EO - 1),
                                )
                        nc.scalar.activation(
                            est[:, g, s0:SL], ps[:, 0:ncols],
                            mybir.ActivationFunctionType.Exp,
                        )
                        nc.vector.tensor_mul(
                            out=est[:, g, s0:s0 + P],
                            in0=est[:, g, s0:s0 + P],
                            in1=maskT[:, g, :],
                        )

                # ---- V projection: v[t,e] = xvT.T @ wvT ----
                with nc.named_scope("projV"):
                    wv = wpool.tile([P, DO, D], fp16, tag="w")
                    for o in range(DO):
                        nc.scalar.dma_start(wv[:, o, :], wv_r[:, o, :])
                    for m in range(GT):
                        xv = xpool.tile([P, DO, P], fp16, tag="xv")
                        nc.sync.dma_start(xv[:], xv_r[:, :, m * P:(m + 1) * P])
                        ps0 = ppsum.tile([P, 512], f32, tag="pp")
                        ps1 = ppsum.tile([P, 512], f32, tag="pp")
                        for o in range(DO):
                            lhsT = xv[:, o, :]
                            nc.tensor.matmul(ps0[:], lhsT, wv[:, o, 0:512],
                                             start=(o == 0), stop=(o == DO - 1))
                            nc.tensor.matmul(ps1[:], lhsT, wv[:, o, 512:1024],
                                             start=(o == 0), stop=(o == DO - 1))
                        nc.any.tensor_copy(vA[:, m, 0:512], ps0[:])
                        nc.any.tensor_copy(vA[:, m, 512:1024], ps1[:])

            # ---- attention-value + row sums + normalize, per s-block j ----
            with nc.named_scope("av"):
                with tc.tile_pool(name="bpsum", bufs=2, space="PSUM") as bpsum:
                    for j in range(JB):
                        gmax = 2 * j + 2
                        ps = bpsum.tile([P, D + 1], f32, tag="bp")
                        for g in range(gmax):
                            lhsT = est[:, g, j * P:(j + 1) * P]
                            st = (g == 0)
                            sp = (g == gmax - 1)
                            # sums column first: on the last g the reciprocal
                            # can overlap the trailing 512-wide matmuls
                            nc.tensor.matmul(ps[:, 1024:1025], lhsT, vA[:, g, 1024:1025],
                                             start=st, stop=sp)
                            nc.tensor.matmul(ps[:, 0:512], lhsT, vA[:, g, 0:512],
                                             start=st, stop=sp)
                            nc.tensor.matmul(ps[:, 512:1024], lhsT, vA[:, g, 512:1024],
                                             start=st, stop=sp)
                        nc.vector.reciprocal(recip[:, j:j + 1], ps[:, D:D + 1])
                        yt = ypool.tile([P, D], fp16, tag="y")
                        for q4 in range(4):
                            c0 = q4 * 256
                            nc.vector.tensor_scalar_mul(
                                yt[:, c0:c0 + 256], ps[:, c0:c0 + 256], recip[:, j:j + 1])
                            eng = nc.sync if q4 % 2 == 0 else nc.scalar
                            eng.dma_start(
                                out_e.ap()[j * P:(j + 1) * P, c0:c0 + 256],
                                yt[:, c0:c0 + 256])

    nc.compile()
    _GRAPH_CACHE["nc"] = nc
    return nc


def _s_index(h):
    return np.concatenate([np.arange(P) + (2 * j + h) * P for j in range(JB)])


def _prepare_in_maps(query, key, value, attn_mask, Wq, Wk, Wv):
    query = np.asarray(query, np.float32)
    key = np.asarray(key, np.float32)
    value = np.asarray(value, np.float32)
    attn_mask = np.asarray(attn_mask)
    Wq = np.asarray(Wq, np.float32)
    Wk = np.asarray(Wk, np.float32)
    Wv = np.asarray(Wv, np.float32)

    scale = np.float32(1.0 / np.sqrt(np.float32(D)))
    wqT = np.ascontiguousarray((Wq * scale).T).astype(np.float16)  # [d, e]
    wkT = np.ascontiguousarray(Wk.T).astype(np.float16)
    wvT = np.ascontiguousarray(Wv.T).astype(np.float16)

    in_maps = []
    for c in range(N_CORES):
        n, h = c // 2, c % 2
        sidx = _s_index(h)
        xqT = np.ascontiguousarray(query[n][sidx].T).astype(np.float16)   # [d, s]
        xkT = np.ascontiguousarray(key[n].T).astype(np.float16)           # [d, t]
        xvT = np.ascontiguousarray(value[n].T).astype(np.float16)         # [d, t]
        maskT = np.empty((GT, P, P), np.float16)
        for g in range(GT):
            j0 = g // 2
            G0 = 2 * j0 + h
            blk = attn_mask[G0 * P:(G0 + 1) * P, g * P:(g + 1) * P]  # [s, t]
            maskT[g] = np.ascontiguousarray(blk.T).astype(np.float16)     # [t, s]
        in_maps.append({
            "xqT": xqT, "xkT": xkT, "xvT": xvT,
            "wqT": wqT, "wkT": wkT, "wvT": wvT, "maskT": maskT,
        })
    return in_maps


def run(trace=False, **inputs):
    nc = _build_graph()
    in_maps = _prepare_in_maps(**inputs)
    res = run_bass_kernel_spmd(nc, in_maps, list(range(N_CORES)), trace=trace)
    out = np.empty((N_BATCH, S, D), np.float32)
    for c in range(N_CORES):
        n, h = c // 2, c % 2
        out[n][_s_index(h)] = res.results[c]["out"].astype(np.float32)
    return out, res


def kernel(**inputs):
    out, _ = run(trace=False, **inputs)
    return out



# revision 2
# speedup vs baseline: 1.1560x; 1.1560x over previous
"""Distributed Trainium2 Bass kernel for a causal single-head attention layer.

Problem shapes (hardcoded): N=4, S=T=2048, D=1024, f32 I/O.
  q = query @ Wq.T ; k = key @ Wk.T ; v = value @ Wv.T
  y = softmax(mask(q k^T / sqrt(D))) v

Sharding over 8 NeuronCores: core c -> (batch n = c//2, parity h = c%2).
Each core owns 8 interleaved 128-row query blocks (global block G = 2j+h,
j=0..7), which balances the causal (triangular) score workload between the
two cores of a batch.

Weight fusion (host side) removes the K and V projections entirely:
  scores: S = (xq Wq^T)(xk Wk^T)^T / sqrt(D) = xq (Wq^T Wk / sqrt(D)) xk^T
          -> precompute Wp = Wq^T Wk / sqrt(D) on host; z = xq @ Wp on
          device (per-core q rows only), then S = z @ xk^T against the RAW
          keys.  No K projection, no duplicated work across the pair.
  values: y = P (xv Wv^T) = (P xv) Wv^T -> AV against RAW values, then a
          1024x1024 post-projection on the core's own output rows only.
Per-core tensor work drops from 7.79G MACs to 4.57G.

Device compute is fp16 (f32 PSUM accumulation); the host pre-transposes/
casts inputs so no on-device input transposes are needed:
  zT[u,s]  = Wp-as-lhsT x xqT          (projZ, baseline projQ structure)
  ST[t,s]  = xkT-tile.T @ zT per 128-wide t-tile g, g <= 2j+1 (causal skip)
  EST      = exp(ST) * mask  (mask data from the real attn_mask input)
  y1[s,:]  = EST.T @ [xv | 1]  (ones-column gives the softmax denominator)
  y1T      = DMA-crossbar transpose of y1 (128x128 fp16 tiles, off-PE)
  out[s,e] = (y1T.T @ WvT) * (1/sums)
"""

import numpy as np

from concourse import bass, mybir, tile, bacc
from concourse.bass_utils import run_bass_kernel_spmd

P = 128
N_BATCH = 4
S = 2048   # full query length
T = 2048   # key/value length
D = 1024   # model dim
SL = 1024  # per-core query rows
JB = SL // P   # 8 local s-blocks per core
GT = T // P    # 16 t-tiles
DO = D // P    # 8 outer tiles of the contraction dim
EO = D // P    # 8 outer tiles of the e dim
N_CORES = 8

_GRAPH_CACHE = {}


def _build_graph():
    if "nc" in _GRAPH_CACHE:
        return _GRAPH_CACHE["nc"]

    fp16 = mybir.dt.float16
    f32 = mybir.dt.float32

    nc = bacc.Bacc("TRN2", target_bir_lowering=False, debug=False,
                   num_devices=N_CORES)

    xqT_e = nc.dram_tensor("xqT", [D, SL], fp16, kind="ExternalInput")
    xkT_e = nc.dram_tensor("xkT", [D, T], fp16, kind="ExternalInput")
    xv_e = nc.dram_tensor("xv", [T, D], fp16, kind="ExternalInput")
    wp_e = nc.dram_tensor("wpT", [D, D], fp16, kind="ExternalInput")
    wv_e = nc.dram_tensor("wvT", [D, D], fp16, kind="ExternalInput")
    mask_e = nc.dram_tensor("maskT", [GT, P, P], fp16, kind="ExternalInput")
    out_e = nc.dram_tensor("out", [SL, D], fp16, kind="ExternalOutput")

    xq_r = xqT_e.ap().rearrange("(o p) s -> p o s", p=P)
    xk_r = xkT_e.ap().rearrange("(o p) t -> p o t", p=P)
    xv_r = xv_e.ap().rearrange("(g p) d -> p g d", p=P)
    wp_r = wp_e.ap().rearrange("(o p) e -> p o e", p=P)
    wv_r = wv_e.ap().rearrange("(o p) e -> p o e", p=P)

    with tile.TileContext(nc) as tc:
        with (
            tc.tile_pool(name="persist", bufs=1) as persist,
            tc.tile_pool(name="weights", bufs=2) as wpool,
            tc.tile_pool(name="xin", bufs=3) as xpool,
            tc.tile_pool(name="yout", bufs=2) as ypool,
            tc.tile_pool(name="trans", bufs=2) as tpool,
        ):
            zT = persist.tile([P, EO, SL], fp16)       # [u-part, u-outer, s]
            xk = persist.tile([P, DO, T], fp16)        # [d-part, d-outer, t]
            xvA = persist.tile([P, GT, D + 1], fp16)   # [t-part, t-outer, d+1]
            est = persist.tile([P, GT, SL], fp16)      # [t-part, t-outer, s]
            maskT = persist.tile([P, GT, P], fp16)     # [t-part, g, s-local]
            wv = persist.tile([P, DO, D], fp16)        # [d-part, d-outer, e]
            recip = persist.tile([P, JB], f32)

            # ---- Z projection: zT[u,s] = Wp.T @ xqT ----
            # First chunk in (o, m) order: accumulate all 8 u-tiles across 8
            # PSUM banks so the first matmul only needs the o=0 input slices.
            with tc.tile_pool(name="qpsum", bufs=8, space="PSUM") as qpsum:
                with nc.named_scope("projZ"):
                    wq = wpool.tile([P, DO, D], fp16, tag="w")
                    nc.scalar.dma_start(wq[:, 0, 0:P], wp_r[:, 0, 0:P])
                    nc.scalar.dma_start(wq[:, 0, P:D], wp_r[:, 0, P:D])
                    for o in range(1, DO):
                        nc.scalar.dma_start(wq[:, o, :], wp_r[:, o, :])
                    xqs = []
                    for sc in range(SL // 512):
                        xq = xpool.tile([P, DO, 512], fp16, tag="x", name=f"xq{sc}")
                        for o in range(DO):
                            if sc == 0 and o == 0:
                                nc.sync.dma_start(xq[:, 0, 0:256], xq_r[:, 0, 0:256])
                                nc.sync.dma_start(xq[:, 0, 256:512], xq_r[:, 0, 256:512])
                            else:
                                nc.sync.dma_start(
                                    xq[:, o, :], xq_r[:, o, 512 * sc:512 * (sc + 1)])
                        xqs.append(xq)
                    # chunk 0: (o, m) order. The o=0 pass only touches columns
                    # 0:256 (so the first matmul needs just the first 64KB DMA
                    # piece); exactly ONE start=True per PSUM bank clears it,
                    # and o=0's contribution to columns 256:512 is accumulated
                    # by a trailing matmul after the o=1..7 full-width passes.
                    pss = [qpsum.tile([P, 512], f32, tag="qp", name=f"qp0_{m2}")
                           for m2 in range(EO)]
                    for m in range(EO):
                        nc.tensor.matmul(
                            pss[m][:, 0:256], wq[:, 0, m * P:(m + 1) * P],
                            xqs[0][:, 0, 0:256],
                            start=True, stop=False, skip_group_check=True,
                        )
                    for o in range(1, DO):
                        for m in range(EO):
                            nc.tensor.matmul(
                                pss[m][:], wq[:, o, m * P:(m + 1) * P], xqs[0][:, o, :],
                                start=False, stop=False, skip_group_check=True,
                            )
                    for m in range(EO):
                        nc.tensor.matmul(
                            pss[m][:, 256:512], wq[:, 0, m * P:(m + 1) * P],
                            xqs[0][:, 0, 256:512],
                            start=False, stop=True, skip_group_check=True,
                        )
                        nc.any.tensor_copy(zT[:, m, 0:512], pss[m][:])
                    # chunk 1: (m, o) order -- psum slots recycle one at a time
                    for m in range(EO):
                        ps = qpsum.tile([P, 512], f32, tag="qp", name=f"qp1_{m}")
                        for o in range(DO):
                            nc.tensor.matmul(
                                ps[:], wq[:, o, m * P:(m + 1) * P], xqs[1][:, o, :],
                                start=(o == 0), stop=(o == DO - 1),
                            )
                        nc.any.tensor_copy(zT[:, m, 512:1024], ps[:])

            # bulk input DMAs for the later phases (issued early; the DMA
            # queues drain them behind projZ compute)
            for o in range(DO):
                nc.sync.dma_start(xk[:, o, :], xk_r[:, o, :])
            for o in range(DO):
                nc.scalar.dma_start(wv[:, o, :], wv_r[:, o, :])
            for g in range(GT):
                nc.scalar.dma_start(xvA[:, g, 0:D], xv_r[:, g, :])
            nc.scalar.dma_start(maskT[:], mask_e.ap().rearrange("g p s -> p g s"))
            nc.vector.memset(xvA[:, :, D:D + 1], 1.0)

            # ---- scores + exp + mask, per t-tile g ----
            with tc.tile_pool(name="spsum", bufs=2, space="PSUM") as spsum:
                with nc.named_scope("scores"):
                    for g in range(GT):
                        j0 = g // 2
                        s0 = j0 * P
                        ncols = SL - s0
                        ps = spsum.tile([P, 1024], f32, tag="sp")
                        n_first = min(512, ncols)
                        for c in range(EO):
                            lhsT = xk[:, c, g * P:(g + 1) * P]
                            nc.tensor.matmul(
                                ps[:, 0:n_first], lhsT, zT[:, c, s0:s0 + n_first],
                                start=(c == 0), stop=(c == EO - 1),
                            )
                            if ncols > 512:
                                nc.tensor.matmul(
                                    ps[:, 512:ncols], lhsT, zT[:, c, s0 + 512:SL],
                                    start=(c == 0), stop=(c == EO - 1),
                                )
                        nc.scalar.activation(
                            est[:, g, s0:SL], ps[:, 0:ncols],
                            mybir.ActivationFunctionType.Exp,
                        )
                        nc.vector.tensor_mul(
                            out=est[:, g, s0:s0 + P],
                            in0=est[:, g, s0:s0 + P],
                            in1=maskT[:, g, :],
                        )

            # ---- attention-value, transpose, post-projection per s-block j ----
            # AV1: y1[s, d+1] = EST.T @ [xv | 1]  (f32 PSUM, causal g range)
            # transpose: 128x128 fp16 tiles via the DMA crossbar (no PE cost)
            # AV2: out[s, e] = y1T.T @ WvT, scaled by 1/sums
            with (
                tc.tile_pool(name="av1psum", bufs=2, space="PSUM") as av1psum,
                tc.tile_pool(name="av2psum", bufs=1, space="PSUM") as av2psum,
            ):
                with nc.named_scope("av"):
                    y1ts = {}

                    def do_av2(j):
                        ps2 = av2psum.tile([P, D], f32, tag="av2")
                        for c in range(DO):
                            lhsT = y1ts.pop((j, c))
                            nc.tensor.matmul(ps2[:, 0:512], lhsT, wv[:, c, 0:512],
                                             start=(c == 0), stop=(c == DO - 1))
                            nc.tensor.matmul(ps2[:, 512:1024], lhsT, wv[:, c, 512:1024],
                                             start=(c == 0), stop=(c == DO - 1))
                        yt = ypool.tile([P, D], fp16, tag="y")
                        for q4 in range(4):
                            c0 = q4 * 256
                            nc.vector.tensor_scalar_mul(
                                yt[:, c0:c0 + 256], ps2[:, c0:c0 + 256],
                                recip[:, j:j + 1])
                            eng = nc.sync if q4 % 2 == 0 else nc.scalar
                            eng.dma_start(
                                out_e.ap()[j * P:(j + 1) * P, c0:c0 + 256],
                                yt[:, c0:c0 + 256])

                    for j in range(JB):
                        gmax = 2 * j + 2
                        ps = av1psum.tile([P, D + 1], f32, tag="av1")
                        for g in range(gmax):
                            lhsT = est[:, g, j * P:(j + 1) * P]
                            st = (g == 0)
                            sp = (g == gmax - 1)
                            # sums column first: on the last g the reciprocal
                            # can overlap the trailing 512-wide matmuls
                            nc.tensor.matmul(ps[:, 1024:1025], lhsT, xvA[:, g, 1024:1025],
                                             start=st, stop=sp)
                            nc.tensor.matmul(ps[:, 0:512], lhsT, xvA[:, g, 0:512],
                                             start=st, stop=sp)
                            nc.tensor.matmul(ps[:, 512:1024], lhsT, xvA[:, g, 512:1024],
                                             start=st, stop=sp)
                        if j >= 2:
                            do_av2(j - 2)
                        nc.vector.reciprocal(recip[:, j:j + 1], ps[:, D:D + 1])
                        for c in range(DO):
                            y1c = tpool.tile([P, P], fp16, tag=f"c{c}")
                            nc.any.tensor_copy(y1c[:], ps[:, c * P:(c + 1) * P])
                            y1t = tpool.tile([P, P], fp16, tag=f"t{c}", bufs=3)
                            nc.sync.dma_start_transpose(y1t[:], y1c[:])
                            y1ts[(j, c)] = y1t
                    do_av2(JB - 2)
                    do_av2(JB - 1)

    nc.compile()
    _GRAPH_CACHE["nc"] = nc
    return nc


def _s_index(h):
    return np.concatenate([np.arange(P) + (2 * j + h) * P for j in range(JB)])


def _prepare_in_maps(query, key, value, attn_mask, Wq, Wk, Wv):
    query = np.asarray(query, np.float32)
    key = np.asarray(key, np.float32)
    value = np.asarray(value, np.float32)
    attn_mask = np.asarray(attn_mask)
    Wq = np.asarray(Wq, np.float32)
    Wk = np.asarray(Wk, np.float32)
    Wv = np.asarray(Wv, np.float32)

    scale = np.float32(1.0 / np.sqrt(np.float32(D)))
    # fused score weight: S = xq @ (Wq.T Wk / sqrt(D)) @ xk.T
    wpT = np.ascontiguousarray((Wq.T @ Wk) * scale).astype(np.float16)  # [d1, d2]
    wvT = np.ascontiguousarray(Wv.T).astype(np.float16)                 # [d, e]

    in_maps = []
    for c in range(N_CORES):
        n, h = c // 2, c % 2
        sidx = _s_index(h)
        xqT = np.ascontiguousarray(query[n][sidx].T).astype(np.float16)   # [d, s]
        xkT = np.ascontiguousarray(key[n].T).astype(np.float16)           # [d, t]
        xv = np.ascontiguousarray(value[n]).astype(np.float16)            # [t, d]
        maskT = np.empty((GT, P, P), np.float16)
        for g in range(GT):
            j0 = g // 2
            G0 = 2 * j0 + h
            blk = attn_mask[G0 * P:(G0 + 1) * P, g * P:(g + 1) * P]  # [s, t]
            maskT[g] = np.ascontiguousarray(blk.T).astype(np.float16)     # [t, s]
        in_maps.append({
            "xqT": xqT, "xkT": xkT, "xv": xv,
            "wpT": wpT, "wvT": wvT, "maskT": maskT,
        })
    return in_maps


def run(trace=False, **inputs):
    nc = _build_graph()
    in_maps = _prepare_in_maps(**inputs)
    res = run_bass_kernel_spmd(nc, in_maps, list(range(N_CORES)), trace=trace)
    out = np.empty((N_BATCH, S, D), np.float32)
    for c in range(N_CORES):
        n, h = c // 2, c % 2
        out[n][_s_index(h)] = res.results[c]["out"].astype(np.float32)
    return out, res


def kernel(**inputs):
    out, _ = run(trace=False, **inputs)
    return out


# revision 5
# speedup vs baseline: 1.4971x; 1.2951x over previous
"""Distributed Trainium2 Bass kernel for a causal single-head attention layer.

Problem shapes (hardcoded): N=4, S=T=2048, D=1024, f32 I/O.
  q = query @ Wq.T ; k = key @ Wk.T ; v = value @ Wv.T
  y = softmax(mask(q k^T / sqrt(D))) v

Sharding over 8 NeuronCores: core c -> (batch n = c//2, parity h = c%2).
Each core owns 8 interleaved 128-row query blocks (global block G = 2j+h,
j=0..7), which balances the causal (triangular) score workload between the
two cores of a batch.

Weight fusion (host side) removes the K and V projections entirely:
  scores: S = (xq Wq^T)(xk Wk^T)^T / sqrt(D) = xq (Wq^T Wk / sqrt(D)) xk^T
          -> precompute Wp = Wq^T Wk / sqrt(D) on host; z = xq @ Wp on
          device (per-core q rows only), then S = z @ xk^T against the RAW
          keys.  No K projection, no duplicated work across the pair.
  values: y = P (xv Wv^T) = (P xv) Wv^T -> AV against RAW values, then a
          1024x1024 post-projection on the core's own output rows only.
Per-core tensor work drops from 7.79G MACs to 4.57G.

Device compute is fp16 (f32 PSUM accumulation); the host pre-transposes/
casts inputs so no on-device input transposes are needed:
  zT[u,s]  = Wp-as-lhsT x xqT          (projZ, baseline projQ structure)
  ST[t,s]  = xkT-tile.T @ zT per 128-wide t-tile g, g <= 2j+1 (causal skip)
  EST      = exp(ST) * mask  (mask data from the real attn_mask input)
  y1[s,:]  = EST.T @ [xv | 1]  (ones-column gives the softmax denominator)
  y1T      = DMA-crossbar transpose of y1 (128x128 fp16 tiles, off-PE)
  out[s,e] = (y1T.T @ WvT) * (1/sums)
"""

import numpy as np

from concourse import bass, mybir, tile, bacc
from concourse.bass_utils import run_bass_kernel_spmd

P = 128
N_BATCH = 4
S = 2048   # full query length
T = 2048   # key/value length
D = 1024   # model dim
SL = 1024  # per-core query rows
JB = SL // P   # 8 local s-blocks per core
GT = T // P    # 16 t-tiles
DO = D // P    # 8 outer tiles of the contraction dim
EO = D // P    # 8 outer tiles of the e dim
N_CORES = 8

_GRAPH_CACHE = {}


def _build_graph():
    if "nc" in _GRAPH_CACHE:
        return _GRAPH_CACHE["nc"]

    fp16 = mybir.dt.float16
    f32 = mybir.dt.float32

    nc = bacc.Bacc("TRN2", target_bir_lowering=False, debug=False,
                   num_devices=N_CORES)

    xqT_e = nc.dram_tensor("xqT", [D, SL], fp16, kind="ExternalInput")
    xkT_e = nc.dram_tensor("xkT", [D, T], fp16, kind="ExternalInput")
    xv_e = nc.dram_tensor("xv", [T, D], fp16, kind="ExternalInput")
    wp_e = nc.dram_tensor("wpT", [D, D], fp16, kind="ExternalInput")
    wv_e = nc.dram_tensor("wvT", [D, D], fp16, kind="ExternalInput")
    mask_e = nc.dram_tensor("maskT", [GT, P, P], fp16, kind="ExternalInput")
    out_e = nc.dram_tensor("out", [SL, D], fp16, kind="ExternalOutput")

    xq_r = xqT_e.ap().rearrange("(o p) s -> p o s", p=P)
    xk_r = xkT_e.ap().rearrange("(o p) t -> p o t", p=P)
    xv_r = xv_e.ap().rearrange("(g p) d -> p g d", p=P)
    wp_r = wp_e.ap().rearrange("(o p) e -> p o e", p=P)
    wv_r = wv_e.ap().rearrange("(o p) e -> p o e", p=P)

    with tile.TileContext(nc) as tc:
        with (
            tc.tile_pool(name="persist", bufs=1) as persist,
            tc.tile_pool(name="weights", bufs=2) as wpool,
            tc.tile_pool(name="xin", bufs=3) as xpool,
            tc.tile_pool(name="yout", bufs=2) as ypool,
            tc.tile_pool(name="trans", bufs=2) as tpool,
        ):
            zT = persist.tile([P, EO, SL], fp16)       # [u-part, u-outer, s]
            xk = persist.tile([P, DO, T], fp16)        # [d-part, d-outer, t]
            xvA = persist.tile([P, GT, D + 1], fp16)   # [t-part, t-outer, d+1]
            est = persist.tile([P, GT, SL], fp16)      # [t-part, t-outer, s]
            maskT = persist.tile([P, GT, P], fp16)     # [t-part, g, s-local]
            wv = persist.tile([P, DO, D], fp16)        # [d-part, d-outer, e]
            recip = persist.tile([P, JB], f32)

            # ---- Z projection: zT[u,s] = Wp.T @ xqT ----
            # First chunk in (o, m) order: accumulate all 8 u-tiles across 8
            # PSUM banks so the first matmul only needs the o=0 input slices.
            with tc.tile_pool(name="qpsum", bufs=8, space="PSUM") as qpsum:
                with nc.named_scope("projZ"):
                    wq = wpool.tile([P, DO, D], fp16, tag="w")
                    nc.scalar.dma_start(wq[:, 0, 0:P], wp_r[:, 0, 0:P])
                    nc.scalar.dma_start(wq[:, 0, P:D], wp_r[:, 0, P:D])
                    for o in range(1, DO):
                        nc.scalar.dma_start(wq[:, o, :], wp_r[:, o, :])
                    xqs = []
                    for sc in range(SL // 512):
                        xq = xpool.tile([P, DO, 512], fp16, tag="x", name=f"xq{sc}")
                        for o in range(DO):
                            if sc == 0 and o == 0:
                                nc.sync.dma_start(xq[:, 0, 0:256], xq_r[:, 0, 0:256])
                                nc.sync.dma_start(xq[:, 0, 256:512], xq_r[:, 0, 256:512])
                            else:
                                nc.sync.dma_start(
                                    xq[:, o, :], xq_r[:, o, 512 * sc:512 * (sc + 1)])
                        xqs.append(xq)
                    # chunk 0: (o, m) order. The o=0 pass only touches columns
                    # 0:256 (so the first matmul needs just the first 64KB DMA
                    # piece); exactly ONE start=True per PSUM bank clears it,
                    # and o=0's contribution to columns 256:512 is accumulated
                    # by a trailing matmul after the o=1..7 full-width passes.
                    pss = [qpsum.tile([P, 512], f32, tag="qp", name=f"qp0_{m2}")
                           for m2 in range(EO)]
                    for m in range(EO):
                        nc.tensor.matmul(
                            pss[m][:, 0:256], wq[:, 0, m * P:(m + 1) * P],
                            xqs[0][:, 0, 0:256],
                            start=True, stop=False, skip_group_check=True,
                        )
                    for o in range(1, DO):
                        for m in range(EO):
                            nc.tensor.matmul(
                                pss[m][:], wq[:, o, m * P:(m + 1) * P], xqs[0][:, o, :],
                                start=False, stop=False, skip_group_check=True,
                            )
                    for m in range(EO):
                        nc.tensor.matmul(
                            pss[m][:, 256:512], wq[:, 0, m * P:(m + 1) * P],
                            xqs[0][:, 0, 256:512],
                            start=False, stop=True, skip_group_check=True,
                        )
                        nc.any.tensor_copy(zT[:, m, 0:512], pss[m][:])
                    # chunk 1: (m, o) order -- psum slots recycle one at a time
                    for m in range(EO):
                        ps = qpsum.tile([P, 512], f32, tag="qp", name=f"qp1_{m}")
                        for o in range(DO):
                            nc.tensor.matmul(
                                ps[:], wq[:, o, m * P:(m + 1) * P], xqs[1][:, o, :],
                                start=(o == 0), stop=(o == DO - 1),
                            )
                        nc.any.tensor_copy(zT[:, m, 512:1024], ps[:])

            # bulk input DMAs for the later phases (issued early; the DMA
            # queues drain them behind projZ compute).  xk is split across
            # both queues by t-half so scores g=0..7 can start as soon as
            # possible after projZ.
            for o in range(DO):
                nc.sync.dma_start(xk[:, o, 0:T // 2], xk_r[:, o, 0:T // 2])
            nc.scalar.dma_start(maskT[:], mask_e.ap().rearrange("g p s -> p g s"))
            for o in range(DO):
                nc.scalar.dma_start(xk[:, o, T // 2:T], xk_r[:, o, T // 2:T])
            for g in range(GT):
                nc.scalar.dma_start(xvA[:, g, 0:D], xv_r[:, g, :])
            for o in range(DO):
                nc.scalar.dma_start(wv[:, o, :], wv_r[:, o, :])
            nc.vector.memset(xvA[:, :, D:D + 1], 1.0)

            # ---- scores + exp + mask, per t-tile g ----
            with tc.tile_pool(name="spsum", bufs=2, space="PSUM") as spsum:
                with nc.named_scope("scores"):
                    for g in range(GT):
                        j0 = g // 2
                        s0 = j0 * P
                        ncols = SL - s0
                        ps = spsum.tile([P, 1024], f32, tag="sp")
                        n_first = min(512, ncols)
                        for c in range(EO):
                            lhsT = xk[:, c, g * P:(g + 1) * P]
                            nc.tensor.matmul(
                                ps[:, 0:n_first], lhsT, zT[:, c, s0:s0 + n_first],
                                start=(c == 0), stop=(c == EO - 1),
                            )
                            if ncols > 512:
                                nc.tensor.matmul(
                                    ps[:, 512:ncols], lhsT, zT[:, c, s0 + 512:SL],
                                    start=(c == 0), stop=(c == EO - 1),
                                )
                        nc.scalar.activation(
                            est[:, g, s0:SL], ps[:, 0:ncols],
                            mybir.ActivationFunctionType.Exp,
                        )
                        nc.vector.tensor_mul(
                            out=est[:, g, s0:s0 + P],
                            in0=est[:, g, s0:s0 + P],
                            in1=maskT[:, g, :],
                        )

            # ---- attention-value, transpose, post-projection per s-block j ----
            # AV1: y1[s, d+1] = EST.T @ [xv | 1]  (f32 PSUM, causal g range)
            # transpose: 128x128 fp16 tiles via the DMA crossbar (no PE cost)
            # AV2: out[s, e] = y1T.T @ WvT, scaled by 1/sums
            with (
                tc.tile_pool(name="av1psum", bufs=2, space="PSUM") as av1psum,
                tc.tile_pool(name="av2psum", bufs=1, space="PSUM") as av2psum,
            ):
                with nc.named_scope("av"):
                    y1ts = {}

                    def do_av2(j):
                        ps2 = av2psum.tile([P, D], f32, tag="av2")
                        y1t = y1ts.pop(j)
                        for c in range(DO):
                            lhsT = y1t[:, c, :]
                            nc.tensor.matmul(ps2[:, 0:512], lhsT, wv[:, c, 0:512],
                                             start=(c == 0), stop=(c == DO - 1))
                            nc.tensor.matmul(ps2[:, 512:1024], lhsT, wv[:, c, 512:1024],
                                             start=(c == 0), stop=(c == DO - 1))
                        yt = ypool.tile([P, D], fp16, tag="y")
                        for q4 in range(4):
                            c0 = q4 * 256
                            nc.vector.tensor_scalar_mul(
                                yt[:, c0:c0 + 256], ps2[:, c0:c0 + 256],
                                recip[:, j:j + 1])
                            eng = nc.sync if q4 % 2 == 0 else nc.scalar
                            eng.dma_start(
                                out_e.ap()[j * P:(j + 1) * P, c0:c0 + 256],
                                yt[:, c0:c0 + 256])

                    for j in range(JB):
                        gmax = 2 * j + 2
                        ps = av1psum.tile([P, D + 1], f32, tag="av1")
                        for g in range(gmax):
                            lhsT = est[:, g, j * P:(j + 1) * P]
                            st = (g == 0)
                            sp = (g == gmax - 1)
                            # sums column first: on the last g the reciprocal
                            # can overlap the trailing 512-wide matmuls
                            nc.tensor.matmul(ps[:, 1024:1025], lhsT, xvA[:, g, 1024:1025],
                                             start=st, stop=sp)
                            nc.tensor.matmul(ps[:, 0:512], lhsT, xvA[:, g, 0:512],
                                             start=st, stop=sp)
                            nc.tensor.matmul(ps[:, 512:1024], lhsT, xvA[:, g, 512:1024],
                                             start=st, stop=sp)
                        if j >= 2:
                            do_av2(j - 2)
                        nc.vector.reciprocal(recip[:, j:j + 1], ps[:, D:D + 1])
                        # one batched crossbar transpose per block: [128,1024]
                        # -> [128, 8, 128] (chunk c = transposed 128x128 tile)
                        y1c = tpool.tile([P, D], fp16, tag="c", name=f"y1c{j}")
                        nc.any.tensor_copy(y1c[:, 0:512], ps[:, 0:512])
                        nc.any.tensor_copy(y1c[:, 512:1024], ps[:, 512:1024])
                        y1t = tpool.tile([P, DO, P], fp16, tag="t", bufs=3,
                                         name=f"y1t{j}")
                        nc.sync.dma_start_transpose(y1t[:], y1c[:])
                        y1ts[j] = y1t
                    do_av2(JB - 2)
                    do_av2(JB - 1)

    nc.compile()
    _GRAPH_CACHE["nc"] = nc
    return nc


def _s_index(h):
    return np.concatenate([np.arange(P) + (2 * j + h) * P for j in range(JB)])


def _prepare_in_maps(query, key, value, attn_mask, Wq, Wk, Wv):
    query = np.asarray(query, np.float32)
    key = np.asarray(key, np.float32)
    value = np.asarray(value, np.float32)
    attn_mask = np.asarray(attn_mask)
    Wq = np.asarray(Wq, np.float32)
    Wk = np.asarray(Wk, np.float32)
    Wv = np.asarray(Wv, np.float32)

    scale = np.float32(1.0 / np.sqrt(np.float32(D)))
    # fused score weight: S = xq @ (Wq.T Wk / sqrt(D)) @ xk.T
    wpT = np.ascontiguousarray((Wq.T @ Wk) * scale).astype(np.float16)  # [d1, d2]
    wvT = np.ascontiguousarray(Wv.T).astype(np.float16)                 # [d, e]

    in_maps = []
    for c in range(N_CORES):
        n, h = c // 2, c % 2
        sidx = _s_index(h)
        xqT = np.ascontiguousarray(query[n][sidx].T).astype(np.float16)   # [d, s]
        xkT = np.ascontiguousarray(key[n].T).astype(np.float16)           # [d, t]
        xv = np.ascontiguousarray(value[n]).astype(np.float16)            # [t, d]
        maskT = np.empty((GT, P, P), np.float16)
        for g in range(GT):
            j0 = g // 2
            G0 = 2 * j0 + h
            blk = attn_mask[G0 * P:(G0 + 1) * P, g * P:(g + 1) * P]  # [s, t]
            maskT[g] = np.ascontiguousarray(blk.T).astype(np.float16)     # [t, s]
        in_maps.append({
            "xqT": xqT, "xkT": xkT, "xv": xv,
            "wpT": wpT, "wvT": wvT, "maskT": maskT,
        })
    return in_maps


def run(trace=False, **inputs):
    nc = _build_graph()
    in_maps = _prepare_in_maps(**inputs)
    res = run_bass_kernel_spmd(nc, in_maps, list(range(N_CORES)), trace=trace)
    out = np.empty((N_BATCH, S, D), np.float32)
    for c in range(N_CORES):
        n, h = c // 2, c % 2
        out[n][_s_index(h)] = res.results[c]["out"].astype(np.float32)
    return out, res


def kernel(**inputs):
    out, _ = run(trace=False, **inputs)
    return out


# revision 8
# speedup vs baseline: 1.5052x; 1.0054x over previous
"""Distributed Trainium2 Bass kernel for a causal single-head attention layer.

Problem shapes (hardcoded): N=4, S=T=2048, D=1024, f32 I/O.
  q = query @ Wq.T ; k = key @ Wk.T ; v = value @ Wv.T
  y = softmax(mask(q k^T / sqrt(D))) v

Sharding over 8 NeuronCores: core c -> (batch n = c//2, parity h = c%2).
Each core owns 8 interleaved 128-row query blocks (global block G = 2j+h,
j=0..7), which balances the causal (triangular) score workload between the
two cores of a batch.

Weight fusion (host side) removes the K and V projections entirely:
  scores: S = (xq Wq^T)(xk Wk^T)^T / sqrt(D) = xq (Wq^T Wk / sqrt(D)) xk^T
          -> precompute Wp = Wq^T Wk / sqrt(D) on host; z = xq @ Wp on
          device (per-core q rows only), then S = z @ xk^T against the RAW
          keys.  No K projection, no duplicated work across the pair.
  values: y = P (xv Wv^T) = (P xv) Wv^T -> AV against RAW values, then a
          1024x1024 post-projection on the core's own output rows only.
Per-core tensor work drops from 7.79G MACs to 4.57G.

Device compute is fp16 (f32 PSUM accumulation); the host pre-transposes/
casts inputs so no on-device input transposes are needed:
  zT[u,s]  = Wp-as-lhsT x xqT          (projZ, baseline projQ structure)
  ST[t,s]  = xkT-tile.T @ zT per 128-wide t-tile g, g <= 2j+1 (causal skip)
  EST      = exp(ST) * mask  (mask data from the real attn_mask input)
  y1[s,:]  = EST.T @ [xv | 1]  (ones-column gives the softmax denominator)
  y1T      = DMA-crossbar transpose of y1 (128x128 fp16 tiles, off-PE)
  out[s,e] = (y1T.T @ WvT) * (1/sums)
"""

import numpy as np

from concourse import bass, mybir, tile, bacc
from concourse.bass_utils import run_bass_kernel_spmd

P = 128
N_BATCH = 4
S = 2048   # full query length
T = 2048   # key/value length
D = 1024   # model dim
SL = 1024  # per-core query rows
JB = SL // P   # 8 local s-blocks per core
GT = T // P    # 16 t-tiles
DO = D // P    # 8 outer tiles of the contraction dim
EO = D // P    # 8 outer tiles of the e dim
N_CORES = 8

_GRAPH_CACHE = {}


def _build_graph():
    if "nc" in _GRAPH_CACHE:
        return _GRAPH_CACHE["nc"]

    fp16 = mybir.dt.float16
    f32 = mybir.dt.float32

    nc = bacc.Bacc("TRN2", target_bir_lowering=False, debug=False,
                   num_devices=N_CORES)

    xqT_e = nc.dram_tensor("xqT", [D, SL], fp16, kind="ExternalInput")
    xkT_e = nc.dram_tensor("xkT", [D, T], fp16, kind="ExternalInput")
    xv_e = nc.dram_tensor("xv", [T, D], fp16, kind="ExternalInput")
    wp_e = nc.dram_tensor("wpT", [D, D], fp16, kind="ExternalInput")
    wv_e = nc.dram_tensor("wvT", [D, D], fp16, kind="ExternalInput")
    mask_e = nc.dram_tensor("maskT", [GT, P, P], fp16, kind="ExternalInput")
    out_e = nc.dram_tensor("out", [SL, D], fp16, kind="ExternalOutput")

    xq_r = xqT_e.ap().rearrange("(o p) s -> p o s", p=P)
    xk_r = xkT_e.ap().rearrange("(o p) t -> p o t", p=P)
    xv_r = xv_e.ap().rearrange("(g p) d -> p g d", p=P)
    wp_r = wp_e.ap().rearrange("(o p) e -> p o e", p=P)
    wv_r = wv_e.ap().rearrange("(o p) e -> p o e", p=P)

    with tile.TileContext(nc) as tc:
        with (
            tc.tile_pool(name="persist", bufs=1) as persist,
            tc.tile_pool(name="weights", bufs=2) as wpool,
            tc.tile_pool(name="xin", bufs=3) as xpool,
            tc.tile_pool(name="yout", bufs=2) as ypool,
            tc.tile_pool(name="trans", bufs=2) as tpool,
        ):
            zT = persist.tile([P, EO, SL], fp16)       # [u-part, u-outer, s]
            xk = persist.tile([P, DO, T], fp16)        # [d-part, d-outer, t]
            xvA = persist.tile([P, GT, D + 1], fp16)   # [t-part, t-outer, d+1]
            est = persist.tile([P, GT, SL], fp16)      # [t-part, t-outer, s]
            maskT = persist.tile([P, GT, P], fp16)     # [t-part, g, s-local]
            wv = persist.tile([P, DO, D], fp16)        # [d-part, d-outer, e]
            recip = persist.tile([P, JB], f32)

            # ---- Z projection: zT[u,s] = Wp.T @ xqT ----
            # First chunk in (o, m) order: accumulate all 8 u-tiles across 8
            # PSUM banks so the first matmul only needs the o=0 input slices.
            with tc.tile_pool(name="qpsum", bufs=8, space="PSUM") as qpsum:
                with nc.named_scope("projZ"):
                    wq = wpool.tile([P, DO, D], fp16, tag="w")
                    nc.scalar.dma_start(wq[:, 0, 0:P], wp_r[:, 0, 0:P])
                    nc.scalar.dma_start(wq[:, 0, P:D], wp_r[:, 0, P:D])
                    for o in range(1, DO):
                        nc.scalar.dma_start(wq[:, o, :], wp_r[:, o, :])
                    xqs = []
                    for sc in range(SL // 512):
                        xq = xpool.tile([P, DO, 512], fp16, tag="x", name=f"xq{sc}")
                        for o in range(DO):
                            if sc == 0 and o == 0:
                                nc.sync.dma_start(xq[:, 0, 0:256], xq_r[:, 0, 0:256])
                                nc.sync.dma_start(xq[:, 0, 256:512], xq_r[:, 0, 256:512])
                            else:
                                nc.sync.dma_start(
                                    xq[:, o, :], xq_r[:, o, 512 * sc:512 * (sc + 1)])
                        xqs.append(xq)
                    # chunk 0: (o, m) order. The o=0 pass only touches columns
                    # 0:256 (so the first matmul needs just the first 64KB DMA
                    # piece); exactly ONE start=True per PSUM bank clears it,
                    # and o=0's contribution to columns 256:512 is accumulated
                    # by a trailing matmul after the o=1..7 full-width passes.
                    pss = [qpsum.tile([P, 512], f32, tag="qp", name=f"qp0_{m2}")
                           for m2 in range(EO)]
                    for m in range(EO):
                        nc.tensor.matmul(
                            pss[m][:, 0:256], wq[:, 0, m * P:(m + 1) * P],
                            xqs[0][:, 0, 0:256],
                            start=True, stop=False, skip_group_check=True,
                        )
                    for o in range(1, DO):
                        for m in range(EO):
                            nc.tensor.matmul(
                                pss[m][:], wq[:, o, m * P:(m + 1) * P], xqs[0][:, o, :],
                                start=False, stop=False, skip_group_check=True,
                            )
                    for m in range(EO):
                        nc.tensor.matmul(
                            pss[m][:, 256:512], wq[:, 0, m * P:(m + 1) * P],
                            xqs[0][:, 0, 256:512],
                            start=False, stop=True, skip_group_check=True,
                        )
                        nc.any.tensor_copy(zT[:, m, 0:512], pss[m][:])
                    # chunk 1: (m, o) order -- psum slots recycle one at a time
                    for m in range(EO):
                        ps = qpsum.tile([P, 512], f32, tag="qp", name=f"qp1_{m}")
                        for o in range(DO):
                            nc.tensor.matmul(
                                ps[:], wq[:, o, m * P:(m + 1) * P], xqs[1][:, o, :],
                                start=(o == 0), stop=(o == DO - 1),
                            )
                        nc.any.tensor_copy(zT[:, m, 512:1024], ps[:])

            # bulk input DMAs for the later phases (issued early; the DMA
            # queues drain them behind projZ compute).  xk is split across
            # both queues by t-half so scores g=0..7 can start as soon as
            # possible after projZ.
            for o in range(DO):
                nc.sync.dma_start(xk[:, o, 0:T // 2], xk_r[:, o, 0:T // 2])
            nc.scalar.dma_start(maskT[:], mask_e.ap().rearrange("g p s -> p g s"))
            for o in range(DO):
                nc.scalar.dma_start(xk[:, o, T // 2:T], xk_r[:, o, T // 2:T])
            for g in range(GT):
                nc.scalar.dma_start(xvA[:, g, 0:D], xv_r[:, g, :])
            for o in range(DO):
                nc.scalar.dma_start(wv[:, o, :], wv_r[:, o, :])
            nc.vector.memset(xvA[:, :, D:D + 1], 1.0)

            # ---- scores + exp + mask, per t-tile g ----
            with tc.tile_pool(name="spsum", bufs=2, space="PSUM") as spsum:
                with nc.named_scope("scores"):
                    for g in range(GT):
                        j0 = g // 2
                        s0 = j0 * P
                        ncols = SL - s0
                        ps = spsum.tile([P, 1024], f32, tag="sp")
                        n_first = min(512, ncols)
                        for c in range(EO):
                            lhsT = xk[:, c, g * P:(g + 1) * P]
                            nc.tensor.matmul(
                                ps[:, 0:n_first], lhsT, zT[:, c, s0:s0 + n_first],
                                start=(c == 0), stop=(c == EO - 1),
                            )
                            if ncols > 512:
                                nc.tensor.matmul(
                                    ps[:, 512:ncols], lhsT, zT[:, c, s0 + 512:SL],
                                    start=(c == 0), stop=(c == EO - 1),
                                )
                        nc.scalar.activation(
                            est[:, g, s0:SL], ps[:, 0:ncols],
                            mybir.ActivationFunctionType.Exp,
                        )
                        nc.vector.tensor_mul(
                            out=est[:, g, s0:s0 + P],
                            in0=est[:, g, s0:s0 + P],
                            in1=maskT[:, g, :],
                        )

            # ---- attention-value, transpose, post-projection per s-block j ----
            # AV1: y1[s, d+1] = EST.T @ [xv | 1]  (f32 PSUM, causal g range)
            # transpose: 128x128 fp16 tiles via the DMA crossbar (no PE cost)
            # AV2: out[s, e] = y1T.T @ WvT, scaled by 1/sums
            with (
                tc.tile_pool(name="av1psum", bufs=2, space="PSUM") as av1psum,
                tc.tile_pool(name="av2psum", bufs=1, space="PSUM") as av2psum,
            ):
                with nc.named_scope("av"):
                    y1ts = {}

                    def do_av2(j):
                        ps2 = av2psum.tile([P, D], f32, tag="av2")
                        y1t = y1ts.pop(j)
                        for c in range(DO):
                            lhsT = y1t[:, c, :]
                            nc.tensor.matmul(ps2[:, 0:512], lhsT, wv[:, c, 0:512],
                                             start=(c == 0), stop=(c == DO - 1))
                            nc.tensor.matmul(ps2[:, 512:1024], lhsT, wv[:, c, 512:1024],
                                             start=(c == 0), stop=(c == DO - 1))
                        yt = ypool.tile([P, D], fp16, tag="y")
                        for q4 in range(4):
                            c0 = q4 * 256
                            nc.vector.tensor_scalar_mul(
                                yt[:, c0:c0 + 256], ps2[:, c0:c0 + 256],
                                recip[:, j:j + 1])
                            # keep the sync queue free for the transposes
                            nc.scalar.dma_start(
                                out_e.ap()[j * P:(j + 1) * P, c0:c0 + 256],
                                yt[:, c0:c0 + 256])

                    for j in range(JB):
                        gmax = 2 * j + 2
                        ps = av1psum.tile([P, D + 1], f32, tag="av1")
                        for g in range(gmax):
                            lhsT = est[:, g, j * P:(j + 1) * P]
                            st = (g == 0)
                            sp = (g == gmax - 1)
                            # sums column first: on the last g the reciprocal
                            # can overlap the trailing 512-wide matmuls
                            nc.tensor.matmul(ps[:, 1024:1025], lhsT, xvA[:, g, 1024:1025],
                                             start=st, stop=sp)
                            nc.tensor.matmul(ps[:, 0:512], lhsT, xvA[:, g, 0:512],
                                             start=st, stop=sp)
                            nc.tensor.matmul(ps[:, 512:1024], lhsT, xvA[:, g, 512:1024],
                                             start=st, stop=sp)
                        if j >= 2:
                            do_av2(j - 2)
                        nc.vector.reciprocal(recip[:, j:j + 1], ps[:, D:D + 1])
                        # one batched crossbar transpose per block: [128,1024]
                        # -> [128, 8, 128] (chunk c = transposed 128x128 tile)
                        y1c = tpool.tile([P, D], fp16, tag="c", name=f"y1c{j}")
                        nc.scalar.copy(y1c[:, 0:512], ps[:, 0:512])
                        nc.vector.tensor_copy(y1c[:, 512:1024], ps[:, 512:1024])
                        y1t = tpool.tile([P, DO, P], fp16, tag="t", bufs=3,
                                         name=f"y1t{j}")
                        nc.sync.dma_start_transpose(y1t[:], y1c[:])
                        y1ts[j] = y1t
                    do_av2(JB - 2)
                    do_av2(JB - 1)

    nc.compile()
    _GRAPH_CACHE["nc"] = nc
    return nc


def _s_index(h):
    return np.concatenate([np.arange(P) + (2 * j + h) * P for j in range(JB)])


def _prepare_in_maps(query, key, value, attn_mask, Wq, Wk, Wv):
    query = np.asarray(query, np.float32)
    key = np.asarray(key, np.float32)
    value = np.asarray(value, np.float32)
    attn_mask = np.asarray(attn_mask)
    Wq = np.asarray(Wq, np.float32)
    Wk = np.asarray(Wk, np.float32)
    Wv = np.asarray(Wv, np.float32)

    scale = np.float32(1.0 / np.sqrt(np.float32(D)))
    # fused score weight: S = xq @ (Wq.T Wk / sqrt(D)) @ xk.T
    wpT = np.ascontiguousarray((Wq.T @ Wk) * scale).astype(np.float16)  # [d1, d2]
    wvT = np.ascontiguousarray(Wv.T).astype(np.float16)                 # [d, e]

    in_maps = []
    for c in range(N_CORES):
        n, h = c // 2, c % 2
        sidx = _s_index(h)
        xqT = np.ascontiguousarray(query[n][sidx].T).astype(np.float16)   # [d, s]
        xkT = np.ascontiguousarray(key[n].T).astype(np.float16)           # [d, t]
        xv = np.ascontiguousarray(value[n]).astype(np.float16)            # [t, d]
        maskT = np.empty((GT, P, P), np.float16)
        for g in range(GT):
            j0 = g // 2
            G0 = 2 * j0 + h
            blk = attn_mask[G0 * P:(G0 + 1) * P, g * P:(g + 1) * P]  # [s, t]
            maskT[g] = np.ascontiguousarray(blk.T).astype(np.float16)     # [t, s]
        in_maps.append({
            "xqT": xqT, "xkT": xkT, "xv": xv,
            "wpT": wpT, "wvT": wvT, "maskT": maskT,
        })
    return in_maps


def run(trace=False, **inputs):
    nc = _build_graph()
    in_maps = _prepare_in_maps(**inputs)
    res = run_bass_kernel_spmd(nc, in_maps, list(range(N_CORES)), trace=trace)
    out = np.empty((N_BATCH, S, D), np.float32)
    for c in range(N_CORES):
        n, h = c // 2, c % 2
        out[n][_s_index(h)] = res.results[c]["out"].astype(np.float32)
    return out, res


def kernel(**inputs):
    out, _ = run(trace=False, **inputs)
    return out
